# revision 8
# baseline (speedup 1.0000x reference)
"""Trainium2 Bass kernel for MergedColumnParallelLinearWithLoRA.

Computes  out = x @ W.T + concat(lora1(x), lora2(x))  where
lora_i(x)[t] = B_i[l_t] @ (A_i[l_t] @ x[t]) + bias_i[l_t],  l_t = indices[t].

Sharding: column-parallel across 8 NeuronCores. Core c owns output columns
[c*704, (c+1)*704) of slice 1 and of slice 2 (1408 columns total). x, lora_a*,
indices are replicated; W / lora_b* / bias* are sharded along the output dim.

Per-core device program:
  - One augmented GEMM  G = x @ [A1_flat; A2_flat; W_slice].T  (fp32r, K=2048)
    produces the LoRA shrink s1|s2 (512 cols) and the base GEMM (1408 cols)
    in one PE stream.
  - Masked dispatch: s_masked = s * (lora_id_col == idx_token), one fused
    scalar_tensor_tensor op per slice; one-hot oh = (iota16 == idx).
  - s_masked|oh transposed via PE (fp32 exact) so the expand can run with
    tokens on PSUM partitions.
  - Expand + bias:  y_i = [s_i_masked | oh] @ [B_i_flat; bias_i]  accumulated
    by the PE directly into the base-GEMM PSUM banks (start=False).
  - PSUM -> SBUF copies split across ScalarE/VectorE, then DMA to DRAM.
"""

import numpy as np

import concourse.bass as bass  # noqa: F401  (engine types referenced via nc)
import concourse.mybir as mybir
import concourse.tile as tile
from concourse import bacc
from concourse.masks import make_identity

T, D, O, L, R = 8192, 2048, 5632, 16, 16
NCORES = 8
OS = O // NCORES  # 704 columns per slice per core
NS = 2 * OS  # 1408 output columns per core
P = 128
KT = D // P  # 16 k-tiles
MT = T // P  # 64 token tiles
SH = 2 * L * R  # 512 shrink columns (s1 | s2)
WA = SH + NS  # 1920 augmented-GEMM columns
F32 = mybir.dt.float32
F32R = mybir.dt.float32r
I32 = mybir.dt.int32

# base-GEMM n-chunks (one PSUM bank each): offsets within the 1408 out cols.
# 4x352 keeps every expand matmul >=256 cols (fp32r full-rate) and aligns
# each chunk to a single LoRA slice.
NCH = [(0, 352), (352, 352), (704, 352), (1056, 352)]


def _expand_mms_for_chunk(off, w):
    """(slice_idx, psum_lo, psum_hi, rhs_lo, rhs_hi) for expand matmuls that
    land inside base chunk [off, off+w)."""
    out = []
    for s, lo, hi in ((0, 0, OS), (1, OS, 2 * OS)):
        a, b = max(off, lo), min(off + w, hi)
        if a < b:
            out.append((s, a - off, b - off, a - lo, b - lo))
    return out


def build_nc(reps=1, mode="full"):
    """mode: 'full' | 'base' (no LoRA) | 'gemm' (augmented GEMM, no expand)."""
    nc = bacc.Bacc("TRN2", target_bir_lowering=False, debug=False)

    xt = nc.dram_tensor("xt", [MT, P, KT, P], F32, kind="ExternalInput")
    wt = nc.dram_tensor("wt", [P, KT, WA], F32, kind="ExternalInput")
    b1 = nc.dram_tensor("b1", [2 * P + L, OS], F32, kind="ExternalInput")
    b2 = nc.dram_tensor("b2", [2 * P + L, OS], F32, kind="ExternalInput")
    idx = nc.dram_tensor("idx", [P, MT], I32, kind="ExternalInput")
    out = nc.dram_tensor("out", [T, NS], F32, kind="ExternalOutput")

    with tile.TileContext(nc) as tc:
        with (
            tc.tile_pool(name="const", bufs=1) as const,
            tc.tile_pool(name="xpool", bufs=3) as xpool,
            tc.tile_pool(name="spool", bufs=2) as spool,
            tc.tile_pool(name="stpool", bufs=2) as stpool,
            tc.tile_pool(name="opool", bufs=2) as opool,
            tc.tile_pool(name="ps_s", bufs=1, space="PSUM") as ps_s,
            tc.tile_pool(name="ps_b", bufs=5, space="PSUM") as ps_b,
            tc.tile_pool(name="ps_t", bufs=1, space="PSUM") as ps_t,
        ):
            # ---------------- resident constants ----------------
            t_w = const.tile([P, KT, WA], F32R, tag="w", name="t_w")
            for kk in range(KT):
                nc.sync.dma_start(t_w[:, kk, :], wt[:, kk, :].bitcast(F32R))

            t_b1 = const.tile([P, 2, OS], F32R, tag="b1", name="t_b1")
            nc.sync.dma_start(
                t_b1[:],
                b1[0 : 2 * P, :].rearrange("(c p) o -> p c o", p=P).bitcast(F32R),
            )
            t_b1c = const.tile([L, OS], F32R, tag="b1c", name="t_b1c")
            nc.sync.dma_start(t_b1c[:], b1[2 * P :, :].bitcast(F32R))
            t_b2 = const.tile([P, 2, OS], F32R, tag="b2", name="t_b2")
            nc.sync.dma_start(
                t_b2[:],
                b2[0 : 2 * P, :].rearrange("(c p) o -> p c o", p=P).bitcast(F32R),
            )
            t_b2c = const.tile([L, OS], F32R, tag="b2c", name="t_b2c")
            nc.sync.dma_start(t_b2c[:], b2[2 * P :, :].bitcast(F32R))

            t_idx = const.tile([P, MT], I32, tag="idxi", name="t_idx")
            nc.sync.dma_start(t_idx[:], idx[:])
            t_idxf = const.tile([P, MT], F32, tag="idxf", name="t_idxf")
            nc.vector.tensor_copy(t_idxf[:], t_idx[:])

            t_identf = const.tile([P, P], F32, tag="identf", name="t_identf")
            make_identity(nc, t_identf[:])
            t_ident = const.tile([P, P], F32R, tag="ident", name="t_ident")
            nc.vector.tensor_copy(t_ident[:], t_identf[:])

            # lora-id per shrink column: col j (within s1 or s2) -> j // R
            t_lidi = const.tile([P, 2, L, R], I32, tag="lidi", name="t_lidi")
            nc.gpsimd.iota(
                t_lidi[:], pattern=[[0, 2], [1, L], [0, R]], base=0, channel_multiplier=0
            )
            t_lid = const.tile([P, SH], F32, tag="lid", name="t_lid")
            nc.vector.tensor_copy(t_lid[:], t_lidi[:].rearrange("p a l r -> p (a l r)"))

            t_i16i = const.tile([P, L], I32, tag="i16i", name="t_i16i")
            nc.gpsimd.iota(t_i16i[:], pattern=[[1, L]], base=0, channel_multiplier=0)
            t_i16 = const.tile([P, L], F32, tag="i16", name="t_i16")
            nc.vector.tensor_copy(t_i16[:], t_i16i[:])

            bmats = (
                (t_b1, t_b1c),
                (t_b2, t_b2c),
            )

            # ---------------- main loop over token tiles ----------------
            for _rep in range(reps):
              for mt in range(MT):
                t_x = xpool.tile([P, KT, P], F32R, tag="x", name="t_x")
                nc.sync.dma_start(t_x[:], xt[mt].bitcast(F32R))

                if mode != "base":
                    # shrink: s1|s2 [128, 512]
                    p_s = ps_s.tile([P, SH], F32, tag="s", name="p_s")
                    for kk in range(KT):
                        nc.tensor.matmul(
                            p_s[:],
                            t_x[:, kk, :],
                            t_w[:, kk, 0:SH],
                            start=(kk == 0),
                            stop=(kk == KT - 1),
                        )

                if mode == "full":
                    # masked dispatch + one-hot, into s_aug [128, 512+16]
                    idx_ap = t_idxf[:, mt : mt + 1]
                    t_sa = spool.tile([P, SH + L], F32R, tag="sa", name="t_sa")
                    nc.vector.scalar_tensor_tensor(
                        t_sa[:, 0:SH],
                        t_lid[:],
                        idx_ap,
                        p_s[:],
                        op0=mybir.AluOpType.is_equal,
                        op1=mybir.AluOpType.mult,
                    )
                    nc.vector.tensor_scalar(
                        t_sa[:, SH : SH + L],
                        t_i16[:],
                        idx_ap,
                        None,
                        op0=mybir.AluOpType.is_equal,
                    )

                    # transpose s_aug via PE: 4x [128,128] + oh [128,16] -> [16,128]
                    p_t = ps_t.tile([P, 5 * P], F32R, tag="t", name="p_t")
                    for j in range(4):
                        nc.tensor.transpose(
                            p_t[:, j * P : (j + 1) * P],
                            t_sa[:, j * P : (j + 1) * P],
                            t_ident[:],
                        )
                    nc.tensor.transpose(
                        p_t[0:L, 4 * P : 5 * P],
                        t_sa[:, SH : SH + L],
                        t_ident[:],
                    )
                    t_st = stpool.tile([P, 5 * P], F32R, tag="st", name="t_st")
                    nc.vector.tensor_copy(t_st[:, 0 : 4 * P], p_t[:, 0 : 4 * P])
                    nc.vector.tensor_copy(
                        t_st[0:L, 4 * P : 5 * P], p_t[0:L, 4 * P : 5 * P]
                    )

                # base GEMM chunks + expand accumulation
                t_out = opool.tile([P, NS], F32, tag="o", name="t_out")
                for ci, (off, w) in enumerate(NCH):
                    p_b = ps_b.tile([P, w], F32, tag="b", name="p_b")
                    do_expand = mode == "full"
                    for kk in range(KT):
                        nc.tensor.matmul(
                            p_b[:],
                            t_x[:, kk, :],
                            t_w[:, kk, SH + off : SH + off + w],
                            start=(kk == 0),
                            stop=(not do_expand and kk == KT - 1),
                        )
                    if do_expand:
                        mms = _expand_mms_for_chunk(off, w)
                        n_mm = 3 * len(mms)
                        i_mm = 0
                        for s, plo, phi, rlo, rhi in mms:
                            t_b, t_bc = bmats[s]
                            for c in range(2):
                                i_mm += 1
                                nc.tensor.matmul(
                                    p_b[:, plo:phi],
                                    t_st[:, (2 * s + c) * P : (2 * s + c + 1) * P],
                                    t_b[:, c, rlo:rhi],
                                    start=False,
                                    stop=(i_mm == n_mm),
                                )
                            i_mm += 1
                            nc.tensor.matmul(
                                p_b[:, plo:phi],
                                t_st[0:L, 4 * P : 5 * P],
                                t_bc[:, rlo:rhi],
                                start=False,
                                stop=(i_mm == n_mm),
                            )
                    # PSUM -> SBUF (alternate engines to balance)
                    if ci % 2 == 1:
                        nc.vector.tensor_copy(t_out[:, off : off + w], p_b[:])
                    else:
                        nc.scalar.copy(t_out[:, off : off + w], p_b[:])
                nc.sync.dma_start(out[mt * P : (mt + 1) * P, :], t_out[:])

    nc.compile()
    return nc


# ---------------------------------------------------------------------------
# host-side sharding / unsharding
# ---------------------------------------------------------------------------


def shard_inputs(x, W, lora_a1, lora_a2, lora_b1, lora_b2, bias1, bias2, indices):
    x = np.asarray(x, np.float32)
    W = np.asarray(W, np.float32)
    indices = np.asarray(indices, np.int32)

    # xt[mt, p, kk, m] = x[mt*128+m, kk*128+p]   (replicated)
    xt = np.ascontiguousarray(x.reshape(MT, P, KT, P).transpose(0, 3, 2, 1))
    idx_t = np.ascontiguousarray(indices.reshape(MT, P).T)

    a1f = np.asarray(lora_a1, np.float32).reshape(L * R, D)
    a2f = np.asarray(lora_a2, np.float32).reshape(L * R, D)

    in_maps = []
    for c in range(NCORES):
        sl = slice(c * OS, (c + 1) * OS)
        w_rows = np.concatenate([W[0:O][sl], W[O : 2 * O][sl]], axis=0)
        w_aug = np.concatenate([a1f, a2f, w_rows], axis=0)  # [1920, 2048]
        wt = np.ascontiguousarray(
            w_aug.T.reshape(KT, P, WA).transpose(1, 0, 2)
        )  # [P, KT, WA]

        def bmat(lb, bias):
            bf = np.asarray(lb, np.float32)[:, sl, :].transpose(0, 2, 1).reshape(
                L * R, OS
            )
            return np.ascontiguousarray(
                np.concatenate([bf, np.asarray(bias, np.float32)[:, sl]], axis=0)
            )

        in_maps.append(
            {
                "xt": xt,
                "wt": wt,
                "b1": bmat(lora_b1, bias1),
                "b2": bmat(lora_b2, bias2),
                "idx": idx_t,
            }
        )
    return in_maps


def unshard_output(results):
    out = np.empty((T, 2 * O), np.float32)
    for c in range(NCORES):
        res = results[c]["out"]
        out[:, c * OS : (c + 1) * OS] = res[:, 0:OS]
        out[:, O + c * OS : O + (c + 1) * OS] = res[:, OS:NS]
    return out


_CACHE = {}


def get_nc():
    if "nc" not in _CACHE:
        _CACHE["nc"] = build_nc()
    return _CACHE["nc"]


def kernel(**inputs):
    from concourse import bass2jax

    nc = get_nc()
    in_maps = shard_inputs(**inputs)
    results = bass2jax.run_bass_via_pjrt(nc, in_maps, n_cores=NCORES)
    return unshard_output(results)
